# revision 50
# baseline (speedup 1.0000x reference)
"""Trainium2 Bass kernel for nn_AttenConv1d (GNN message passing attention).

Per node n (batch b):
  x_i = x[b, idx1[n,:]]   [16,128]   (centers)
  x_j = x[b, idx0[n,:]]   [16,128]   (neighbors)
  S = x_i @ x_j.T / sqrt(128)        [16,16]
  P = softmax(S, -1)
  h = (P @ x_j).sum(0)               [128]
  y = relu((x[b,n] + h) @ W.T + b)

8 cores: core c handles batch c//4, node slice (c%4)*4096. This problem is
tunnel-transfer-bound (axon H2D ~68MB/s, D2H ~29MB/s), so the pipeline is
built to minimize host<->device bytes:
  - x ships once as bf16 node shards (1MB/core); a jax prelude all-gathers
    the full 32768-row two-batch table on-device and feeds it directly as
    the bass kernel's table parameter (device-resident, no replication over
    the tunnel). mask/b1/W^T/bias-broadcast consts are also built on-device.
  - indices ship un-replicated as [16, nch, 264] i16 and are broadcast to
    the 128-partition wrapped layout dma_gather wants with 8 on-device DMAs.
    Layout per chunk of 128 nodes: [XI 2048 | XJ 2048 | OWN 128] tokens;
    the OWN block doubles as the residual (replaces a PE transpose of a
    separately-shipped xown tensor), and the XJ block doubles as the row
    gather index list (replaces a separate idxr tensor).
  - y returns as bf16 (halves the slow D2H) and is upcast on host.
On-chip per core: bf16 table [128, 256, 128] in SBUF; dma_gather(transpose)
for score columns, DRAM row gather for values; groups of 8 nodes = 128
(node,k) pairs fill the partition dim; block-diagonal bf16 score matmul,
masked exp softmax with fused row-sum, two small matmuls per group, fused
final linear.
"""

import concurrent.futures as cf
import math
import sys

import numpy as np

for _p in ("/opt/trn_rl_repo",):
    if _p not in sys.path:
        sys.path.insert(0, _p)

import jax
import jax.numpy as jnp
import ml_dtypes
from jax.sharding import Mesh, PartitionSpec as P

try:
    from jax.experimental.shard_map import shard_map
except ImportError:
    from jax.shard_map import shard_map

import concourse.bass as bass
import concourse.bacc as bacc
import concourse.mybir as mybir
from concourse import bass2jax, library_config, tile

B, N, K, C = 2, 16384, 128 // 8, 128  # K=16
CORES = 8
TOTN = B * N                  # 32768 rows in the fused two-batch table
NPC = TOTN // CORES           # nodes per core = 4096
CHUNK = 128                   # nodes per chunk
NCH = NPC // CHUNK            # chunks per core = 32
G = 16                        # groups per chunk (8 nodes each)
GN = CHUNK // G               # nodes per group = 8
NTOK = 2 * CHUNK * G + CHUNK  # gathered col tokens per chunk = 4224
SCALE = 1.0 / math.sqrt(C)

f32 = mybir.dt.float32
bf16 = mybir.dt.bfloat16
i16 = mybir.dt.int16


def build_nc():
    nc = bacc.Bacc("TRN2", target_bir_lowering=False, debug=False,
                   num_swdge_queues=2, num_devices=CORES)
    # per-core shard of the packed x/scale/W/b int8 stream; AllGather'd
    # in-kernel so the whole pipeline is a single execution
    xsh = nc.dram_tensor(
        "xsh", [OFF_IDX // CORES, 128], mybir.dt.int8, kind="ExternalInput"
    ).ap()
    idxw = nc.dram_tensor("idxw", [16, NCH, NTOK // 16], i16, kind="ExternalInput").ap()
    maskneg = nc.dram_tensor("maskneg", [128, 128], f32, kind="ExternalInput").ap()
    b1 = nc.dram_tensor("b1", [128, GN], f32, kind="ExternalInput").ap()
    identb = nc.dram_tensor("identb", [128, 128], bf16, kind="ExternalInput").ap()
    # y ships int8 with a per-node scale (relu output, rowmax/127 quant):
    # halves the slow D2H vs bf16 at ~0.4%-of-max worst-case error. The f32
    # scale rides in the last 4 columns so the fetch is a single array.
    y = nc.dram_tensor("y", [NPC, C + 4], mybir.dt.int8, kind="ExternalOutput").ap()

    NR = TOTN // 128  # 256 table ranks
    NW = NTOK // 16   # 264 wrapped index columns
    i8 = mybir.dt.int8

    with tile.TileContext(nc) as tc:
        nc.gpsimd.load_library(library_config.mlp)
        with (
            tc.tile_pool(name="dram", bufs=1, space="DRAM") as dpool,
            tc.tile_pool(name="const", bufs=1) as cpool,
            tc.tile_pool(name="gath", bufs=2) as gpool,
            tc.tile_pool(name="work", bufs=3) as wpool,
            tc.tile_pool(name="tiny", bufs=4) as tpool,
            tc.tile_pool(name="psS", bufs=2, space="PSUM") as psS,
            tc.tile_pool(name="psT", bufs=2, space="PSUM") as psT,
            tc.tile_pool(name="psW", bufs=1, space="PSUM") as psW,
            tc.tile_pool(name="psZ", bufs=1, space="PSUM") as psZ,
            tc.tile_pool(name="psY", bufs=1, space="PSUM") as psY,
        ):
            # ---- all-gather the packed stream (collectives can't touch
            # I/O tensors, so bounce through internal DRAM) ----
            xb = dpool.tile([OFF_IDX // CORES, 128], i8, tag="xb")
            xfull = dpool.tile([OFF_IDX, 128], i8, tag="xfull")
            nc.sync.dma_start(out=xb[:], in_=xsh)
            nc.gpsimd.collective_compute(
                "AllGather",
                mybir.AluOpType.bypass,
                replica_groups=[list(range(CORES))],
                ins=[xb[:].opt()],
                outs=[xfull[:].opt()],
            )
            xfv = xfull[:]

            # ---- persistent constants / tables ----
            # int8 x table + wrapped scales -> dequantized bf16 table
            xq_sb = cpool.tile([128, NR, C], i8, tag="xq")
            xq_v = xfv[:TOTN].rearrange("(r t) c -> t r c", t=128)
            for rb in range(0, NR, NR // 8):
                nc.gpsimd.dma_start(
                    out=xq_sb[:, rb : rb + NR // 8, :],
                    in_=xq_v[:, rb : rb + NR // 8, :],
                )
            scw = cpool.tile([128, NR], f32, tag="scw")
            nc.sync.dma_start(
                out=scw[:],
                in_=xfv[OFF_SC : OFF_SC + S8]
                .rearrange("(p e) c -> p (e c)", p=128)
                .bitcast(f32),
            )
            table = cpool.tile([128, NR, C], bf16, tag="table")
            for r in range(NR):
                nc.vector.tensor_scalar_mul(
                    table[:, r, :], xq_sb[:, r, :], scw[:, r : r + 1]
                )
            # indices: broadcast 16-partition wrapped layout to all 128
            idx_sb = cpool.tile([128, NCH, NW], i16, tag="idx")
            for rep in range(8):
                nc.sync.dma_start(out=idx_sb[16 * rep : 16 * (rep + 1), :, :], in_=idxw)
            mask_sb = cpool.tile([128, 128], f32, tag="mask")
            nc.sync.dma_start(out=mask_sb[:], in_=maskneg)
            b1_sb = cpool.tile([128, GN], f32, tag="b1")
            nc.sync.dma_start(out=b1_sb[:], in_=b1)
            id_sb = cpool.tile([128, 128], bf16, tag="identb")
            nc.sync.dma_start(out=id_sb[:], in_=identb)
            # W^T (f32) from the packed bf16 W rows via PE transpose
            Wb_sb = cpool.tile([128, 2 * C], i8, tag="Wb")
            nc.sync.dma_start(
                out=Wb_sb[:],
                in_=xfv[OFF_W : OFF_W + W8].rearrange("(o e) c -> o (e c)", o=128),
            )
            wtp = psT.tile([128, 128], bf16, tag="xjt")
            nc.tensor.transpose(wtp[:], Wb_sb[:].bitcast(bf16), id_sb[:])
            wt_sb = cpool.tile([C, C], f32, tag="wt")
            nc.vector.tensor_copy(wt_sb[:], wtp[:])
            # bias broadcast [128, C] from the packed bf16 b row via ones @ b
            b_sb = cpool.tile([1, 2 * C], i8, tag="brow")
            nc.sync.dma_start(
                out=b_sb[:],
                in_=xfv[OFF_B : OFF_B + B8].rearrange("(o e) c -> o (e c)", o=1),
            )
            ones_sb = cpool.tile([1, 128], bf16, tag="ones")
            nc.vector.memset(ones_sb[:], 1.0)
            bbp = psY.tile([128, C], f32, tag="yps")
            nc.tensor.matmul(
                bbp[:], lhsT=ones_sb[:], rhs=b_sb[:].bitcast(bf16),
                start=True, stop=True,
            )
            bbc_sb = cpool.tile([128, C], f32, tag="bbc")
            nc.vector.tensor_copy(bbc_sb[:], bbp[:])

            table_raw = table[:].rearrange("p r c -> p (r c)")

            GSZ = 896  # max idxs per dma_gather instruction (1024 crashes HW)

            def _chunks(total):
                o = 0
                while o < total:
                    n = min(GSZ, total - o)
                    yield o, n
                    o += n

            for ch in range(NCH):
                # gathered bf16 columns: [:, :2048]=XI, [:, 2048:4096]=XJ,
                # [:, 4096:4224]=OWN (residual x for this chunk's nodes)
                cols = gpool.tile([128, 1, NTOK], bf16, tag="cols")
                for qi, (o, n) in enumerate(_chunks(NTOK)):
                    nc.gpsimd.dma_gather(
                        out_ap=cols[:, :, o : o + n],
                        in_ap=table_raw,
                        idxs_ap=idx_sb[:, ch, o // 16 : (o + n) // 16],
                        num_idxs=n,
                        num_idxs_reg=n,
                        elem_size=C,
                        transpose=True,
                        sbuf_tokens_per_rank=128,
                        sbuf_free_dim_per_rank=2 * C,
                        queue_num=qi % 2,
                    )
                colsv = cols[:].rearrange("p one n -> p (one n)")

                zps = psZ.tile([128, CHUNK], f32, tag="zps")
                for g in range(G):
                    # x_j rows for aggregation: PE transpose of the gathered
                    # XJ columns (replaces a second dma_gather of rows)
                    xjt = psT.tile([128, 128], bf16, tag="xjt")
                    nc.tensor.transpose(
                        xjt[:], colsv[:, 2048 + g * 128 : 2048 + (g + 1) * 128],
                        id_sb[:],
                    )
                    xjs = wpool.tile([128, 128], bf16, tag="xjs")
                    nc.vector.tensor_copy(xjs[:], xjt[:])
                    ps = psS.tile([128, 128], f32, tag="ps")
                    nc.tensor.matmul(
                        ps[:],
                        lhsT=colsv[:, g * 128 : (g + 1) * 128],
                        rhs=colsv[:, 2048 + g * 128 : 2048 + (g + 1) * 128],
                        start=True,
                        stop=True,
                    )
                    ms = wpool.tile([128, 128], f32, tag="ms")
                    nc.vector.tensor_add(ms[:], ps[:], mask_sb[:])
                    E = wpool.tile([128, 128], bf16, tag="E")
                    Z = tpool.tile([128, 1], f32, tag="Z")
                    nc.scalar.activation(
                        E[:], ms[:], mybir.ActivationFunctionType.Exp,
                        scale=SCALE, accum_out=Z[:],
                    )
                    R = tpool.tile([128, 1], f32, tag="R")
                    nc.vector.reciprocal(R[:], Z[:])
                    b1r = tpool.tile([128, GN], bf16, tag="b1r")
                    nc.vector.tensor_scalar_mul(b1r[:], b1_sb[:], R[:])
                    pw = psW.tile([128, GN], f32, tag="pw")
                    nc.tensor.matmul(pw[:], lhsT=E[:], rhs=b1r[:], start=True, stop=True)
                    wm = tpool.tile([128, GN], bf16, tag="wm")
                    nc.vector.tensor_copy(wm[:], pw[:])
                    nc.tensor.matmul(
                        zps[:, g * GN : (g + 1) * GN],
                        lhsT=xjs[:],
                        rhs=wm[:],
                        start=True,
                        stop=True,
                    )

                # z = x_own^T + h^T : OWN cols block is the residual
                ownf = wpool.tile([128, CHUNK], f32, tag="ownf")
                nc.vector.tensor_copy(ownf[:], colsv[:, 4096:4224])
                zsb = wpool.tile([128, CHUNK], f32, tag="zsb")
                nc.vector.tensor_add(zsb[:], zps[:], ownf[:])
                yps = psY.tile([128, C], f32, tag="yps")
                nc.tensor.matmul(yps[:], lhsT=zsb[:], rhs=wt_sb[:], start=True, stop=True)
                ysb = wpool.tile([128, C], f32, tag="ysb")
                nc.vector.tensor_add(ysb[:], yps[:], bbc_sb[:])
                yr = wpool.tile([128, C], f32, tag="yr")
                nc.scalar.activation(yr[:], ysb[:], mybir.ActivationFunctionType.Relu)
                rmax = tpool.tile([128, 1], f32, tag="rmax")
                nc.vector.tensor_reduce(
                    rmax[:], yr[:], mybir.AxisListType.X, mybir.AluOpType.max
                )
                rsc = tpool.tile([128, 1], f32, tag="rsc")
                nc.vector.tensor_scalar(
                    rsc[:], rmax[:], 1.0 / 127.0, 1e-30,
                    mybir.AluOpType.mult, mybir.AluOpType.max,
                )
                rs = tpool.tile([128, 1], f32, tag="rs")
                nc.vector.reciprocal(rs[:], rsc[:])
                yq = wpool.tile([128, C + 4], mybir.dt.int8, tag="yq")
                nc.vector.tensor_scalar_mul(yq[:, :C], yr[:], rs[:])
                nc.vector.tensor_copy(yq[:, C:], rsc[:].bitcast(mybir.dt.int8))
                nc.sync.dma_start(out=y[ch * 128 : (ch + 1) * 128, :], in_=yq[:])
    nc.compile()
    return nc


# packed int8 H2D stream [PROWS, 128]: x int8 (per-node scales), scale f32
# bytes, W bf16 bytes, b bf16 bytes, pad, idx int16 bytes
X8 = TOTN                     # 32768 x rows, int8 per-row-quantized
S8 = TOTN * 4 // 128          # 1024 rows of f32 scale bytes
W8 = C * C * 2 // 128         # 256 rows of W bf16 bytes
B8 = C * 2 // 128             # 2 rows of b bf16 bytes
PAD8 = 6                      # align idx start to /8 total
IDX8 = CORES * 16 * NCH * (NTOK // 16) * 2 // 128  # 16896 rows of idx bytes
OFF_SC = X8
OFF_W = OFF_SC + S8
OFF_B = OFF_W + W8
OFF_IDX = OFF_B + B8 + PAD8   # 34056
PROWS = OFF_IDX + IDX8        # 50952, /8 per-core shards


def make_idx(edge_index):
    """Global wrapped index tensor [8*16, NCH, 264] i16.

    Per core: tokens per chunk of 128 nodes = [e1(2048) | e0(2048) | own(128)],
    each +16384 for batch-1 cores (fused two-batch table), wrapped so token t
    sits at (partition t%16, column t//16). Core c = batch c//4, slice c%4,
    which is exactly row-major order of the [2, 4, ...] reshape."""
    e = np.asarray(edge_index)
    offs = (np.arange(CORES, dtype=e.dtype) // 4 * N)[:, None, None]
    e1 = e[1].reshape(CORES, NCH, CHUNK * G) + offs
    e0 = e[0].reshape(CORES, NCH, CHUNK * G) + offs
    own = np.broadcast_to(
        np.arange(TOTN, dtype=e.dtype).reshape(CORES, NCH, CHUNK), e1[..., :CHUNK].shape
    )
    a = np.concatenate([e1, e0, own], axis=2).astype(np.int16)  # [8, NCH, 4224]
    w = a.reshape(CORES, NCH, NTOK // 16, 16).transpose(0, 3, 1, 2)
    return np.ascontiguousarray(w.reshape(CORES * 16, NCH, NTOK // 16))


_CACHE = {}


def _setup():
    bass2jax.install_neuronx_cc_hook()
    nc = build_nc()
    assert nc.dbg_addr is None
    devs = jax.devices()[:CORES]
    mesh = Mesh(np.asarray(devs), ("core",))

    in_names, out_names, out_avals = [], [], []
    for alloc in nc.m.functions[0].allocations:
        if not isinstance(alloc, mybir.MemoryLocationSet):
            continue
        name = alloc.memorylocations[0].name
        if alloc.kind == "ExternalInput":
            if nc.partition_id_tensor is None or name != nc.partition_id_tensor.name:
                in_names.append(name)
        elif alloc.kind == "ExternalOutput":
            out_names.append(name)
            out_avals.append(
                jax.core.ShapedArray(tuple(alloc.tensor_shape), mybir.dt.np(alloc.dtype))
            )
    n_params, n_outs = len(in_names), len(out_names)
    pname = nc.partition_id_tensor.name if nc.partition_id_tensor else None
    all_in = tuple(in_names) + ((pname,) if pname else ())

    def _body(*args):
        operands = list(args)
        if pname is not None:
            operands.append(bass2jax.partition_id_tensor())
        outs = bass2jax._bass_exec_p.bind(
            *operands,
            out_avals=tuple(out_avals),
            in_names=all_in,
            out_names=tuple(out_names),
            lowering_input_output_aliases=(),
            sim_require_finite=True,
            sim_require_nnan=True,
            nc=nc,
        )
        return tuple(outs)

    run = jax.jit(
        shard_map(
            _body, mesh=mesh,
            in_specs=(P("core"),) * n_params,
            out_specs=(P("core"),) * n_outs,
            check_rep=False,
        ),
        keep_unused=True,
    )

    zerosF = jax.jit(
        shard_map(
            lambda: jnp.zeros((NPC, C + 4), jnp.int8), mesh=mesh,
            in_specs=(), out_specs=P("core"), check_rep=False,
        )
    )

    def _consts():
        i = jnp.arange(128)
        mask = jnp.where(
            (i[:, None] // K) == (i[None, :] // K), 0.0, -1e9
        ).astype(jnp.float32)
        b1m = ((i[:, None] // K) == jnp.arange(GN)[None, :]).astype(jnp.float32)
        ident = jnp.eye(128, dtype=jnp.bfloat16)
        return mask, b1m, ident

    constF = jax.jit(
        shard_map(
            _consts, mesh=mesh, in_specs=(), out_specs=(P("core"),) * 3,
            check_rep=False,
        )
    )
    maskD, b1D, identD = constF()
    from jax.sharding import NamedSharding
    _CACHE.update(
        nc=nc, run=run, zerosF=zerosF, in_names=in_names, out_names=out_names,
        maskD=maskD, b1D=b1D, identD=identD,
        shc8=NamedSharding(mesh, P("core")),
        pool=cf.ThreadPoolExecutor(CORES),
    )


def kernel(x, edge_index, W, b, **kw):
    if "run" not in _CACHE:
        _setup()
    idx_fut = _CACHE["pool"].submit(lambda: make_idx(edge_index))
    x2 = np.asarray(x, dtype=np.float32).reshape(TOTN, C)
    xg = np.empty((OFF_IDX, 128), np.int8)
    sc = np.empty(TOTN, np.float32)

    def _quant(i):
        lo, hi = i * (TOTN // 8), (i + 1) * (TOTN // 8)
        blk = x2[lo:hi]
        s = np.maximum(np.abs(blk).max(1), 1e-30) * (1.0 / 127.0)
        sc[lo:hi] = s
        xg[lo:hi] = np.rint(blk * (1.0 / s)[:, None])

    list(_CACHE["pool"].map(_quant, range(8)))
    # scales packed wrapped [partition t, rank r] = sc[r*128 + t] to match
    # the kernel's [128, NR] f32 DMA layout
    xg[OFF_SC : OFF_SC + S8] = (
        np.ascontiguousarray(sc.reshape(TOTN // 128, 128).T)
        .view(np.int8).reshape(S8, 128)
    )
    xg[OFF_W : OFF_W + W8] = (
        np.asarray(W, dtype=np.float32).astype(ml_dtypes.bfloat16)
        .view(np.int8).reshape(W8, 128)
    )
    xg[OFF_B : OFF_B + B8] = (
        np.asarray(b, dtype=np.float32).astype(ml_dtypes.bfloat16)
        .view(np.int8).reshape(B8, 128)
    )
    xg[OFF_B + B8 : OFF_IDX] = 0
    xD = jax.device_put(xg, _CACHE["shc8"])  # x H2D streams while idx builds
    idxg = idx_fut.result()
    args = {"xsh": xD, "idxw": idxg, "maskneg": _CACHE["maskD"],
            "b1": _CACHE["b1D"], "identb": _CACHE["identD"]}
    outs = _CACHE["run"](*[args[n] for n in _CACHE["in_names"]])
    yD = outs[0]
    y = np.empty((B, N, C), np.float32)
    yv = y.reshape(CORES, NPC, C)

    def _fetch(i, shy):
        q = np.asarray(shy.data)  # [NPC, C+4] int8
        s = np.ascontiguousarray(q[:, C:]).view(np.float32)  # [NPC, 1]
        yv[i] = q[:, :C].astype(np.float32) * s

    try:
        ysh = sorted(yD.addressable_shards, key=lambda s: s.index[0].start or 0)
        assert len(ysh) == CORES
        for sh in ysh:
            try:
                sh.data.copy_to_host_async()
            except Exception:
                pass
        list(_CACHE["pool"].map(lambda t: _fetch(*t), enumerate(ysh)))
    except Exception:
        q = np.asarray(yD).reshape(CORES, NPC, C + 4)
        s = np.ascontiguousarray(q[:, :, C:]).view(np.float32).reshape(CORES, NPC, 1)
        yv[:] = q[:, :, :C].astype(np.float32) * s
    return y.reshape(B, N, C)
